# revision 1
# baseline (speedup 1.0000x reference)
"""GCN (4-layer GCNConv + BN + global mean/max pool + MLP) on 8 Trainium2 cores.

Strategy (node sharding, per the graph-partitioning hint):
  - Nodes are re-laid-out graph-aligned: each core owns 8 of the 64 graphs,
    each graph padded to a multiple of 128 node slots, so every 128-node SBUF
    block belongs to exactly one graph.
  - Each core owns every edge whose dst lands in its node range.  Edges are
    host-sorted by (dst block, src half) and padded to a uniform per-(block,
    half) chunk cap so all 8 cores run one SPMD program.
  - Per layer: h@W + dinv row-scale produce the scaled feature table u
    (bf16); an AllGather replicates the table; per-edge rows are fetched with
    dma_gather; per-128-edge chunks are summed into per-block PSUM via a
    one-hot (dst==node) matmul; BatchNorm stats go through a tiny AllGather.
  - GCN normalization uses the identity
        conv = dinv * (sum_{e->n} u[src] + u[n]) + b,   u = dinv * (h @ W)
    so no per-edge norm multiply is needed.
  - Pooling: per-block one-hot(graph) matmul for mean+counts; masked
    block-max reduction for max; one AllGather combines across cores; the
    tiny MLP runs redundantly on every core.
"""

import os
import numpy as np
import ml_dtypes

import concourse.bass as bass
import concourse.bacc as bacc
import concourse.tile as tile
from concourse import mybir
from concourse.masks import make_identity

P = 128
NC = 8
EPS = 1e-5
F32 = mybir.dt.float32
BF16 = mybir.dt.bfloat16
I16 = mybir.dt.int16
I32 = mybir.dt.int32
AF = mybir.ActivationFunctionType
ALU = mybir.AluOpType
BF16NP = ml_dtypes.bfloat16

PROFILE = False
SIM = False  # run MultiCoreSim instead of hardware (dev only)
TAPS = False  # dump per-layer intermediates (dev only)
BISECT = set()  # dev: {"noagg"} drop gather/M path, {"nocoll"} drop collectives
LAST_RESULTS = {}


def _install_ntff_hook_shim():
    """Provide antenv.axon_hooks when the agent image lacks it, so
    run_bass_kernel_spmd(trace=True) can capture NTFF profiles."""
    import sys
    import types
    import ctypes
    import contextlib
    if "antenv.axon_hooks" in sys.modules:
        return
    so = "/opt/axon/libaxon_pjrt.so"
    if not os.path.exists(so):
        return
    try:
        lib = ctypes.CDLL(so)
    except OSError:
        return
    if not hasattr(lib, "axon_start_nrt_profile"):
        return
    lib.axon_start_nrt_profile.argtypes = [
        ctypes.POINTER(ctypes.c_int64), ctypes.c_size_t]
    lib.axon_start_nrt_profile.restype = ctypes.c_int64
    lib.axon_stop_nrt_profile.argtypes = [ctypes.c_char_p]
    lib.axon_stop_nrt_profile.restype = ctypes.c_int64

    @contextlib.contextmanager
    def _hook(output_dir, device_ids):
        import jax
        jax.devices()
        if device_ids:
            ids = (ctypes.c_int64 * len(device_ids))(*device_ids)
            rc = lib.axon_start_nrt_profile(ids, len(device_ids))
        else:
            rc = lib.axon_start_nrt_profile(None, 0)
        if rc != 0:
            raise RuntimeError(f"axon_start_nrt_profile rc={rc}")
        try:
            yield
        finally:
            n = lib.axon_stop_nrt_profile(str(output_dir).encode())
            if n <= 0:
                print(f"ntff profile: {n} files written", flush=True)

    mod = types.ModuleType("antenv.axon_hooks")
    mod.get_axon_ntff_profile_hook = lambda: _hook
    mod.set_axon_ntff_profile_hook = lambda h: None
    sys.modules["antenv.axon_hooks"] = mod


# ----------------------------------------------------------------------------
# Host-side preprocessing: integer graph partitioning / layout only.
# ----------------------------------------------------------------------------
def _prep(x, edge_index, batch):
    N, F = x.shape
    E = edge_index.shape[1]
    G = 64  # fixed number of graph segments in this problem
    GPC = (G + NC - 1) // NC

    batch = np.asarray(batch).astype(np.int64)
    src = np.asarray(edge_index[0]).astype(np.int64)
    dst = np.asarray(edge_index[1]).astype(np.int64)

    counts = np.bincount(batch, minlength=G)
    nb = np.maximum((counts + P - 1) // P, 0)  # blocks per graph
    core_of_g = np.minimum(np.arange(G) // GPC, NC - 1)
    B = 0
    for c in range(NC):
        B = max(B, int(nb[core_of_g == c].sum()))
    NODES_PC = B * P
    NTOT = NC * NODES_PC
    HALF = NTOT // 2
    assert HALF < 32768, f"gather index range too large: {HALF}"

    # stable order of nodes by graph (batch is sorted in the reference, but
    # don't rely on it)
    perm = np.argsort(batch, kind="stable")
    cum = np.concatenate([[0], np.cumsum(counts)])

    # new node id for each old node
    old2new = np.empty(N, dtype=np.int64)
    blk_graph = np.full((NC, B), -1, dtype=np.int64)  # graph id per block
    for c in range(NC):
        off = 0  # block offset within core
        for g in range(c * GPC, min((c + 1) * GPC, G)):
            n_g = int(counts[g])
            ids = perm[cum[g]:cum[g] + n_g]
            base = c * NODES_PC + off * P
            old2new[ids] = base + np.arange(n_g)
            blk_graph[c, off:off + int(nb[g])] = g
            off += int(nb[g])

    # per-node arrays in new layout
    x_new = np.zeros((NTOT, F), dtype=np.float32)
    x_new[old2new] = np.asarray(x, dtype=np.float32)
    deg = np.bincount(dst, minlength=N).astype(np.float32)
    deg_new = np.zeros(NTOT, dtype=np.float32)
    deg_new[old2new] = deg
    bid_new = np.full(NTOT, -1.0, dtype=np.float32)
    bid_new[old2new] = batch.astype(np.float32)
    rm_new = np.zeros(NTOT, dtype=np.float32)
    rm_new[old2new] = 1.0

    def per_core_pb(a):  # [NTOT] -> [NC][128, B]
        return a.reshape(NC, B, P).transpose(0, 2, 1).copy()

    # graphmask [NC][1, G*B]: 1 if block b of core c belongs to graph g
    gmask = np.zeros((NC, 1, G * B), dtype=np.float32)
    for c in range(NC):
        for b in range(B):
            g = blk_graph[c, b]
            if g >= 0:
                gmask[c, 0, g * B + b] = 1.0

    # ---- edges ----
    src_n = old2new[src]
    dst_n = old2new[dst]
    core_e = dst_n // NODES_PC
    lb = (dst_n % NODES_PC) // P
    dl = dst_n % P
    half = (src_n >= HALF).astype(np.int64)
    idxv = (src_n - half * HALF).astype(np.int64)

    key = (core_e * B + lb) * 2 + half
    order = np.argsort(key, kind="stable")
    key_s = key[order]
    idx_s = idxv[order]
    dl_s = dl[order]
    grp_cnt = np.bincount(key_s, minlength=NC * B * 2)
    K = int((grp_cnt.max() + P - 1) // P)
    CAP = K * P

    idx_pad = np.zeros((NC * B * 2, CAP), dtype=np.int16)
    dl_pad = np.full((NC * B * 2, CAP), -1.0, dtype=np.float32)
    starts = np.concatenate([[0], np.cumsum(grp_cnt)])
    flat_pos = (np.arange(E) - starts[key_s]) + key_s * CAP
    idx_pad.reshape(-1)[flat_pos] = idx_s.astype(np.int16)
    dl_pad.reshape(-1)[flat_pos] = dl_s.astype(np.float32)

    idx_pad = idx_pad.reshape(NC, B, 2, CAP)
    dl_pad = dl_pad.reshape(NC, B, 2, CAP)

    def wrap_idx(stream):  # [L] int16 -> [128, L//16]
        L = stream.shape[0]
        w = stream.reshape(L // 16, 16).T
        return np.tile(w, (8, 1)).copy()

    idx_w = np.empty((NC, 2, P, B * CAP // 16), dtype=np.int16)
    dl_t = np.empty((NC, 2, P, B * K), dtype=BF16NP)
    for c in range(NC):
        for h in range(2):
            stream = idx_pad[c, :, h, :].reshape(-1)
            idx_w[c, h] = wrap_idx(stream)
            dl_t[c, h] = dl_pad[c, :, h, :].reshape(B * K, P).T.astype(BF16NP)

    cfg = dict(N=N, F=F, G=G, B=B, K=K, NODES_PC=NODES_PC, NTOT=NTOT, HALF=HALF)
    percore = dict(
        x=[x_new.reshape(NC, NODES_PC, F)[c].astype(BF16NP) for c in range(NC)],
        deg=list(per_core_pb(deg_new)),
        batchid=list(per_core_pb(bid_new)),
        realmask=list(per_core_pb(rm_new)),
        graphmask=[gmask[c] for c in range(NC)],
        idx_lo=[idx_w[c, 0] for c in range(NC)],
        idx_hi=[idx_w[c, 1] for c in range(NC)],
        dstl_lo=[dl_t[c, 0] for c in range(NC)],
        dstl_hi=[dl_t[c, 1] for c in range(NC)],
    )
    return cfg, percore


# ----------------------------------------------------------------------------
# Bass program
# ----------------------------------------------------------------------------
def _build(cfg, GRP=4):
    B, K, G = cfg["B"], cfg["K"], cfg["G"]
    NODES_PC, NTOT, HALF = cfg["NODES_PC"], cfg["NTOT"], cfg["HALF"]
    F = cfg["F"]
    NREAL = cfg["N"]  # real node count for BN mean
    CAP = K * P
    L = B * CAP  # edge slots per half

    nc = bacc.Bacc("TRN2", target_bir_lowering=False, debug=False,
                   num_devices=NC)

    din = {}
    def dram_in(name, shape, dt):
        din[name] = nc.dram_tensor(name, shape, dt, kind="ExternalInput")
        return din[name]

    x_d = dram_in("x", [NODES_PC, F], BF16)
    deg_d = dram_in("deg", [P, B], F32)
    bid_d = dram_in("batchid", [P, B], F32)
    rm_d = dram_in("realmask", [P, B], F32)
    gm_d = dram_in("graphmask", [1, G * B], F32)
    ixlo_d = dram_in("idx_lo", [P, L // 16], I16)
    ixhi_d = dram_in("idx_hi", [P, L // 16], I16)
    dllo_d = dram_in("dstl_lo", [P, B * K], BF16)
    dlhi_d = dram_in("dstl_hi", [P, B * K], BF16)
    W_d = dram_in("W", [4, F, F], BF16)
    lw1_d = dram_in("lw1", [2 * F, F], F32)
    lw2_d = dram_in("lw2", [F, 16], F32)  # padded to 16 cols
    b4_d = dram_in("b4", [1, F], F32)
    gam_d = dram_in("gamma", [3, F], F32)
    bet_d = dram_in("beta", [3, F], F32)
    lb1_d = dram_in("lb1", [1, F], F32)
    lb2_d = dram_in("lb2", [1, 16], F32)
    out_d = nc.dram_tensor("out", [64, 16], F32, kind="ExternalOutput")
    taps = {}
    if TAPS:
        for k in range(4):
            taps[f"tap_u{k}"] = nc.dram_tensor(
                f"tap_u{k}", [P, B * F], BF16, kind="ExternalOutput")
            taps[f"tap_conv{k}"] = nc.dram_tensor(
                f"tap_conv{k}", [P, B * F], F32, kind="ExternalOutput")
            taps[f"tap_h{k}"] = nc.dram_tensor(
                f"tap_h{k}", [P, B * F], F32, kind="ExternalOutput")
        for k in range(3):
            taps[f"tap_stats{k}"] = nc.dram_tensor(
                f"tap_stats{k}", [NC, 2 * F], F32, kind="ExternalOutput")
            taps[f"tap_coef{k}"] = nc.dram_tensor(
                f"tap_coef{k}", [P, 2 * F], F32, kind="ExternalOutput")
        taps["tap_dinv"] = nc.dram_tensor(
            "tap_dinv", [P, B], F32, kind="ExternalOutput")
        taps["tap_part"] = nc.dram_tensor(
            "tap_part", [64, 2 * F + 1], F32, kind="ExternalOutput")

    NGRP = (B + GRP - 1) // GRP

    with tile.TileContext(nc) as tc:
        with (
            tc.tile_pool(name="dram", bufs=1, space="DRAM") as dram,
            tc.tile_pool(name="persist", bufs=1) as ps,
            tc.tile_pool(name="work", bufs=2) as wk,
            tc.tile_pool(name="hTp", bufs=3) as hTp,
            tc.tile_pool(name="gbufp", bufs=2) as gbufp,
            tc.tile_pool(name="mbufp", bufs=1) as mbufp,
            tc.tile_pool(name="ps_tr", bufs=1, space="PSUM") as ps_tr,
            tc.tile_pool(name="ps_t", bufs=1, space="PSUM") as ps_t,
            tc.tile_pool(name="ps_agg", bufs=2, space="PSUM") as ps_agg,
            tc.tile_pool(name="ps_stats", bufs=1, space="PSUM") as ps_stats,
            tc.tile_pool(name="ps_pool", bufs=1, space="PSUM") as ps_pool,
            tc.tile_pool(name="ps_misc", bufs=1, space="PSUM") as ps_misc,
        ):
            # --- internal DRAM ---
            u_own = dram.tile([NODES_PC, F], BF16)
            tables = [dram.tile([NTOT, F], BF16, addr_space="Shared",
                                name=f"table{j}", tag=f"table{j}")
                      for j in range(4)]
            table_hi = dram.tile([HALF, F], BF16)
            stats_in = dram.tile([1, 2 * F], F32)
            stats_outs = [dram.tile([NC, 2 * F], F32, addr_space="Shared",
                                    name=f"stats_out{j}", tag=f"stats_out{j}")
                          for j in range(3)]
            pool_in = dram.tile([64, 2 * F + 1], F32)
            pool_out = dram.tile([NC * 64, 2 * F + 1], F32, addr_space="Shared")

            # --- persistent SBUF ---
            ident_b = ps.tile([P, P], BF16)
            make_identity(nc, ident_b[:])
            ident_f = ps.tile([P, P], F32)
            make_identity(nc, ident_f[:])
            iota_i = ps.tile([P, P], I32)
            nc.gpsimd.iota(iota_i[:], pattern=[[1, P]], base=0,
                           channel_multiplier=0)
            iota_b = ps.tile([P, P], BF16)
            nc.vector.tensor_copy(iota_b[:], iota_i[:])
            giota_i = ps.tile([P, 64], I32)
            nc.gpsimd.iota(giota_i[:], pattern=[[1, 64]], base=0,
                           channel_multiplier=0)
            giota_f = ps.tile([P, 64], F32)
            nc.vector.tensor_copy(giota_f[:], giota_i[:])
            ones_row = ps.tile([1, P], F32)
            nc.vector.memset(ones_row[:], 1.0)
            ones8 = ps.tile([NC, 1], F32)
            nc.vector.memset(ones8[:], 1.0)

            deg_sb = ps.tile([P, B], F32)
            nc.sync.dma_start(out=deg_sb[:], in_=deg_d[:, :])
            dinv = ps.tile([P, B], F32)
            nc.vector.tensor_scalar_add(out=dinv[:], in0=deg_sb[:], scalar1=1.0)
            nc.vector.reciprocal(out=dinv[:], in_=dinv[:])
            nc.scalar.activation(dinv[:], dinv[:], AF.Sqrt)
            if TAPS:
                nc.sync.dma_start(out=taps["tap_dinv"][:, :], in_=dinv[:])
            bid_sb = ps.tile([P, B], F32)
            nc.sync.dma_start(out=bid_sb[:], in_=bid_d[:, :])
            rm_sb = ps.tile([P, B], F32)
            nc.sync.dma_start(out=rm_sb[:], in_=rm_d[:, :])
            gm_sb = ps.tile([1, G * B], F32)
            nc.sync.dma_start(out=gm_sb[:], in_=gm_d[:, :])
            ixlo_sb = ps.tile([P, L // 16], I16)
            nc.sync.dma_start(out=ixlo_sb[:], in_=ixlo_d[:, :])
            ixhi_sb = ps.tile([P, L // 16], I16)
            nc.sync.dma_start(out=ixhi_sb[:], in_=ixhi_d[:, :])
            dllo_sb = ps.tile([P, B * K], BF16)
            nc.sync.dma_start(out=dllo_sb[:], in_=dllo_d[:, :])
            dlhi_sb = ps.tile([P, B * K], BF16)
            nc.sync.dma_start(out=dlhi_sb[:], in_=dlhi_d[:, :])
            b4row = ps.tile([1, F], F32)
            nc.sync.dma_start(out=b4row[:], in_=b4_d[:, :])
            lb1row = ps.tile([1, F], F32)
            nc.sync.dma_start(out=lb1row[:], in_=lb1_d[:, :])
            lb2row = ps.tile([1, 16], F32)
            nc.sync.dma_start(out=lb2row[:], in_=lb2_d[:, :])

            h_sb = ps.tile([P, B * F], BF16)
            nc.sync.dma_start(
                out=h_sb[:].rearrange("p (b f) -> p b f", b=B),
                in_=x_d[:, :].rearrange("(b p) f -> p b f", p=P),
            )
            conv_sb = ps.tile([P, B * F], F32)
            u_sb = ps.tile([P, B * F], BF16)

            # b4 broadcast [128, F] via outer product
            b4_ps = ps_misc.tile([P, 512], F32, tag="misc")
            nc.tensor.matmul(b4_ps[:, 0:F], lhsT=ones_row[:], rhs=b4row[:],
                             start=True, stop=True)
            b4_bc = ps.tile([P, F], F32)
            nc.scalar.activation(b4_bc[:], b4_ps[:, 0:F], AF.Copy)

            # ---------------- layers ----------------
            for k in range(4):
                W_sb = wk.tile([P, F], BF16, tag="W")
                nc.sync.dma_start(out=W_sb[:], in_=W_d[k, :, :])

                # phase A: u = dinv * (h @ W) per block; stage into u_sb + DRAM
                for b in range(B):
                    hT_ps = ps_tr.tile([P, P], BF16, tag="trps")
                    nc.tensor.transpose(hT_ps[:], h_sb[:, b * F:(b + 1) * F],
                                        ident_b[:])
                    hT = hTp.tile([P, P], BF16, tag="hT")
                    nc.scalar.activation(hT[:], hT_ps[:], AF.Copy)
                    t_ps = ps_t.tile([P, F], F32, tag="tps")
                    nc.tensor.matmul(t_ps[:], lhsT=hT[:], rhs=W_sb[:],
                                     start=True, stop=True)
                    nc.scalar.activation(u_sb[:, b * F:(b + 1) * F], t_ps[:],
                                         AF.Copy, scale=dinv[:, b:b + 1])
                    nc.sync.dma_start(out=u_own[b * P:(b + 1) * P, :],
                                      in_=u_sb[:, b * F:(b + 1) * F])

                if TAPS:
                    nc.sync.dma_start(out=taps[f"tap_u{k}"][:, :], in_=u_sb[:])
                # phase B: replicate the table
                table = tables[k]
                if "nocoll" in BISECT:
                    nc.sync.dma_start(out=table[0:NODES_PC, :], in_=u_own[:, :])
                else:
                    nc.gpsimd.collective_compute(
                        "AllGather", ALU.bypass,
                        replica_groups=[list(range(NC))],
                        ins=[u_own[:, :].opt()],
                        outs=[table[:, :].opt()],
                    )
                # dma_gather ignores in_ap row offsets on HW; stage the upper
                # half at a base address of its own
                nc.sync.dma_start(out=table_hi[:, :], in_=table[HALF:NTOT, :])

                # phase C: aggregate into conv
                if k < 3:
                    st_mu = ps_stats.tile([1, F], F32, tag="stats")
                    st_e2 = ps_pool.tile([1, F], F32, tag="pool")
                for g in range(NGRP):
                    b0 = g * GRP
                    nb = min(GRP, B - b0)
                    nidx = nb * CAP
                    if "noagg" in BISECT:
                        for bb in range(nb):
                            b = b0 + bb
                            agg = ps_agg.tile([P, F], F32, tag="agg")
                            nc.tensor.matmul(agg[:], lhsT=ident_b[:],
                                             rhs=u_sb[:, b * F:(b + 1) * F],
                                             start=True, stop=True)
                            nc.scalar.activation(conv_sb[:, b * F:(b + 1) * F],
                                                 agg[:], AF.Copy,
                                                 scale=dinv[:, b:b + 1])
                            if k < 3:
                                sq = wk.tile([P, F], F32, tag="sq")
                                nc.vector.tensor_tensor(
                                    out=sq[:], in0=conv_sb[:, b * F:(b + 1) * F],
                                    in1=conv_sb[:, b * F:(b + 1) * F], op=ALU.mult)
                                nc.tensor.matmul(st_mu[:, :], lhsT=rm_sb[:, b:b + 1],
                                                 rhs=conv_sb[:, b * F:(b + 1) * F],
                                                 start=(b == 0), stop=(b == B - 1),
                                                 skip_group_check=True)
                                nc.tensor.matmul(st_e2[:, :],
                                                 lhsT=rm_sb[:, b:b + 1], rhs=sq[:],
                                                 start=(b == 0), stop=(b == B - 1),
                                                 skip_group_check=True)
                        continue
                    gl = gbufp.tile([P, nb * K, P], BF16, tag="gl")
                    gh = gbufp.tile([P, nb * K, P], BF16, tag="gh")
                    if "nogather" in BISECT:
                        nc.vector.memset(gl[:], 0.0)
                        nc.vector.memset(gh[:], 0.0)
                    else:
                        nc.gpsimd.dma_gather(
                            out_ap=gl[:], in_ap=table[0:HALF, :],
                            idxs_ap=ixlo_sb[:, b0 * CAP // 16:(b0 * CAP + nidx) // 16],
                            num_idxs=nidx, num_idxs_reg=nidx, elem_size=F,
                            single_packet=False)
                        nc.gpsimd.dma_gather(
                            out_ap=gh[:], in_ap=table_hi[:, :],
                            idxs_ap=ixhi_sb[:, b0 * CAP // 16:(b0 * CAP + nidx) // 16],
                            num_idxs=nidx, num_idxs_reg=nidx, elem_size=F,
                            single_packet=False)
                    ml = mbufp.tile([P, nb * K, P], BF16, tag="ml")
                    mh = mbufp.tile([P, nb * K, P], BF16, tag="mh")
                    nc.vector.tensor_tensor(
                        out=ml[:], in0=dllo_sb[:, b0 * K:(b0 + nb) * K]
                        .to_broadcast([P, nb * K, P]),
                        in1=iota_b[:].rearrange("p (o f) -> p o f", o=1)
                        .to_broadcast([P, nb * K, P]),
                        op=ALU.is_equal)
                    nc.vector.tensor_tensor(
                        out=mh[:], in0=dlhi_sb[:, b0 * K:(b0 + nb) * K]
                        .to_broadcast([P, nb * K, P]),
                        in1=iota_b[:].rearrange("p (o f) -> p o f", o=1)
                        .to_broadcast([P, nb * K, P]),
                        op=ALU.is_equal)
                    for bb in range(nb):
                        b = b0 + bb
                        agg = ps_agg.tile([P, F], F32, tag="agg")
                        for j in range(K):
                            nc.tensor.matmul(
                                agg[:], lhsT=ml[:, bb * K + j, :],
                                rhs=gl[:, bb * K + j, :],
                                start=(j == 0), stop=False)
                        for j in range(K):
                            nc.tensor.matmul(
                                agg[:], lhsT=mh[:, bb * K + j, :],
                                rhs=gh[:, bb * K + j, :],
                                start=False, stop=False)
                        # self loop term
                        nc.tensor.matmul(agg[:], lhsT=ident_b[:],
                                         rhs=u_sb[:, b * F:(b + 1) * F],
                                         start=False, stop=True)
                        nc.scalar.activation(conv_sb[:, b * F:(b + 1) * F],
                                             agg[:], AF.Copy,
                                             scale=dinv[:, b:b + 1])
                        if k < 3:
                            sq = wk.tile([P, F], F32, tag="sq")
                            nc.vector.tensor_tensor(
                                out=sq[:], in0=conv_sb[:, b * F:(b + 1) * F],
                                in1=conv_sb[:, b * F:(b + 1) * F], op=ALU.mult)
                            nc.tensor.matmul(st_mu[:, :], lhsT=rm_sb[:, b:b + 1],
                                             rhs=conv_sb[:, b * F:(b + 1) * F],
                                             start=(b == 0), stop=(b == B - 1),
                                             skip_group_check=True)
                            nc.tensor.matmul(st_e2[:, :],
                                             lhsT=rm_sb[:, b:b + 1], rhs=sq[:],
                                             start=(b == 0), stop=(b == B - 1),
                                             skip_group_check=True)

                if TAPS:
                    nc.sync.dma_start(out=taps[f"tap_conv{k}"][:, :], in_=conv_sb[:])
                if k < 3:
                    # phase D: BN stats allreduce (via allgather) + apply + relu
                    st_sb = wk.tile([1, 2 * F], F32, tag="strow")
                    nc.scalar.activation(st_sb[:, 0:F], st_mu[:, :], AF.Copy)
                    nc.scalar.activation(st_sb[:, F:2 * F], st_e2[:, :], AF.Copy)
                    nc.sync.dma_start(out=stats_in[:, :], in_=st_sb[:])
                    stats_out = stats_outs[k]
                    if "nocoll" in BISECT:
                        nc.sync.dma_start(out=stats_out[0:1, :], in_=stats_in[:, :])
                    else:
                        nc.gpsimd.collective_compute(
                            "AllGather", ALU.bypass,
                            replica_groups=[list(range(NC))],
                            ins=[stats_in[:, :].opt()],
                            outs=[stats_out[:, :].opt()],
                        )
                    srows = wk.tile([NC, 2 * F], F32, tag="srows")
                    nc.sync.dma_start(out=srows[:], in_=stats_out[:, :])
                    if TAPS:
                        nc.sync.dma_start(out=taps[f"tap_stats{k}"][:, :],
                                          in_=srows[:])
                    tot_ps = ps_misc.tile([P, 512], F32, tag="misc")
                    nc.tensor.matmul(tot_ps[0:1, 0:2 * F], lhsT=ones8[:],
                                     rhs=srows[:], start=True, stop=True)
                    # mean | E[x^2]
                    mrow = wk.tile([1, 2 * F], F32, tag="mrow")
                    nc.scalar.activation(mrow[:], tot_ps[0:1, 0:2 * F],
                                         AF.Copy, scale=1.0 / NREAL)
                    coef = wk.tile([1, 2 * F], F32, tag="coef")
                    # var = E[x^2] - mean^2  -> coef[0:F] (temp)
                    nc.vector.tensor_tensor(out=coef[:, 0:F],
                                            in0=mrow[:, 0:F], in1=mrow[:, 0:F],
                                            op=ALU.mult)
                    nc.vector.tensor_tensor(out=coef[:, 0:F],
                                            in0=mrow[:, F:2 * F],
                                            in1=coef[:, 0:F], op=ALU.subtract)
                    # rs = rsqrt(var + eps)
                    nc.vector.tensor_scalar_add(out=coef[:, 0:F],
                                                in0=coef[:, 0:F], scalar1=EPS)
                    nc.vector.reciprocal(out=coef[:, 0:F], in_=coef[:, 0:F])
                    nc.scalar.activation(coef[:, 0:F], coef[:, 0:F], AF.Sqrt)
                    grow = wk.tile([1, F], F32, tag="grow")
                    nc.sync.dma_start(out=grow[:], in_=gam_d[k:k + 1, :])
                    brow = wk.tile([1, F], F32, tag="brow")
                    nc.sync.dma_start(out=brow[:], in_=bet_d[k:k + 1, :])
                    # scale = gamma * rs
                    nc.vector.tensor_tensor(out=coef[:, 0:F], in0=coef[:, 0:F],
                                            in1=grow[:], op=ALU.mult)
                    # shift = beta - mean * scale
                    tmp = wk.tile([1, F], F32, tag="tmprow")
                    nc.vector.tensor_tensor(out=tmp[:], in0=mrow[:, 0:F],
                                            in1=coef[:, 0:F], op=ALU.mult)
                    nc.vector.tensor_tensor(out=coef[:, F:2 * F], in0=brow[:],
                                            in1=tmp[:], op=ALU.subtract)
                    cf_ps = ps_misc.tile([P, 512], F32, tag="misc")
                    nc.tensor.matmul(cf_ps[:, 0:2 * F], lhsT=ones_row[:],
                                     rhs=coef[:], start=True, stop=True)
                    cf_bc = wk.tile([P, 2 * F], F32, tag="cfbc")
                    nc.scalar.activation(cf_bc[:], cf_ps[:, 0:2 * F], AF.Copy)
                    if TAPS:
                        nc.sync.dma_start(out=taps[f"tap_coef{k}"][:, :],
                                          in_=cf_bc[:])
                    # conv = conv*scale + shift ; h = relu(conv) (bf16)
                    nc.vector.tensor_tensor(
                        out=conv_sb[:], in0=conv_sb[:],
                        in1=cf_bc[:, 0:F].rearrange("p (o f) -> p o f", o=1)
                        .to_broadcast([P, B, F]), op=ALU.mult)
                    nc.vector.tensor_tensor(
                        out=conv_sb[:], in0=conv_sb[:],
                        in1=cf_bc[:, F:2 * F].rearrange("p (o f) -> p o f", o=1)
                        .to_broadcast([P, B, F]), op=ALU.add)
                    nc.scalar.activation(h_sb[:], conv_sb[:], AF.Relu)
                    if TAPS:
                        nc.gpsimd.dma_start(out=taps[f"tap_h{k}"][:, :], in_=h_sb[:])
                else:
                    # layer 4: conv += b4 ; h4 = relu(conv) kept f32 in conv_sb
                    nc.vector.tensor_tensor(
                        out=conv_sb[:], in0=conv_sb[:],
                        in1=b4_bc[:].rearrange("p (o f) -> p o f", o=1)
                        .to_broadcast([P, B, F]), op=ALU.add)
                    nc.scalar.activation(conv_sb[:], conv_sb[:], AF.Relu)
                    if TAPS:
                        nc.sync.dma_start(out=taps[f"tap_h{k}"][:, :], in_=conv_sb[:])

            # ---------------- pooling ----------------
            bmax = ps.tile([P, B], F32)
            pool_s = ps_pool.tile([64, F], F32, tag="pool")
            pool_c = ps_stats.tile([64, 1], F32, tag="stats")
            for b in range(B):
                # mask out padding rows (relu output >= 0 so max is safe)
                nc.vector.tensor_scalar_mul(
                    out=conv_sb[:, b * F:(b + 1) * F],
                    in0=conv_sb[:, b * F:(b + 1) * F],
                    scalar1=rm_sb[:, b:b + 1])
                S = wk.tile([P, 64], F32, tag="S")
                nc.vector.tensor_tensor(
                    out=S[:], in0=bid_sb[:, b:b + 1].to_broadcast([P, 64]),
                    in1=giota_f[:], op=ALU.is_equal)
                nc.tensor.matmul(pool_s[:, :], lhsT=S[:],
                                 rhs=conv_sb[:, b * F:(b + 1) * F],
                                 start=(b == 0), stop=(b == B - 1),
                                 skip_group_check=True)
                nc.tensor.matmul(pool_c[:, :], lhsT=S[:],
                                 rhs=rm_sb[:, b:b + 1],
                                 start=(b == 0), stop=(b == B - 1),
                                 skip_group_check=True)
                trf = ps_tr.tile([P, P], F32, tag="trf")
                nc.tensor.transpose(trf[:], conv_sb[:, b * F:(b + 1) * F],
                                    ident_f[:])
                h4T = hTp.tile([P, P], F32, tag="h4T")
                nc.scalar.activation(h4T[:], trf[:], AF.Copy)
                nc.vector.tensor_reduce(out=bmax[:, b:b + 1], in_=h4T[:],
                                        axis=mybir.AxisListType.X, op=ALU.max)
            # per-graph max via masked block-max, chunked by graph groups
            gmaxT = ps.tile([P, 64], F32)
            GG = max(1, 512 // B)  # graphs per chunk so GG*B <= 512
            for g0 in range(0, G, GG):
                g1 = min(g0 + GG, G)
                w = (g1 - g0) * B
                mk_ps = ps_misc.tile([P, 512], F32, tag="misc")
                nc.tensor.matmul(mk_ps[:, 0:w], lhsT=ones_row[:],
                                 rhs=gm_sb[:, g0 * B:g1 * B],
                                 start=True, stop=True)
                mck = wk.tile([P, GG * B], F32, tag="mck")
                nc.vector.tensor_tensor(
                    out=mck[:, 0:w],
                    in0=bmax[:].rearrange("p (o b) -> p o b", o=1)
                    .to_broadcast([P, g1 - g0, B]),
                    in1=mk_ps[:, 0:w].rearrange("p (g b) -> p g b", b=B),
                    op=ALU.mult)
                nc.vector.tensor_reduce(
                    out=gmaxT[:, g0:g1],
                    in_=mck[:, 0:w].rearrange("p (g b) -> p g b", b=B),
                    axis=mybir.AxisListType.X, op=ALU.max)
            if G < 64:
                nc.vector.memset(gmaxT[:, G:64], 0.0)
            # partials -> [64, 257]: [sum | max | count]
            part = ps.tile([64, 2 * F + 1], F32)
            nc.scalar.activation(part[:, 0:F], pool_s[:, :], AF.Copy)
            nc.scalar.activation(part[:, 2 * F:2 * F + 1], pool_c[:, :],
                                 AF.Copy)
            gm_ps = ps_misc.tile([P, 512], F32, tag="misc")
            nc.tensor.transpose(gm_ps[0:64, 0:P], gmaxT[:], ident_f[:])
            nc.scalar.activation(part[:, F:2 * F], gm_ps[0:64, 0:P], AF.Copy)
            nc.sync.dma_start(out=pool_in[:, :], in_=part[:])
            if TAPS:
                nc.sync.dma_start(out=taps["tap_part"][:, :], in_=part[:])
            if "nocoll" in BISECT:
                nc.sync.dma_start(out=pool_out[0:64, :], in_=pool_in[:, :])
            else:
                nc.gpsimd.collective_compute(
                    "AllGather", ALU.bypass,
                    replica_groups=[list(range(NC))],
                    ins=[pool_in[:, :].opt()],
                    outs=[pool_out[:, :].opt()],
                )
            DP = 2 * F + 1
            pall = ps.tile([64, DP * NC], F32)
            nc.sync.dma_start(
                out=pall[:].rearrange("g (d c) -> g d c", c=NC),
                in_=pool_out[:, :].rearrange("(c g) d -> g d c", c=NC),
            )
            red = ps.tile([64, DP], F32)
            nc.vector.tensor_reduce(
                out=red[:, 0:F],
                in_=pall[:].rearrange("g (d c) -> g d c", c=NC)[:, 0:F, :],
                axis=mybir.AxisListType.X, op=ALU.add)
            nc.vector.tensor_reduce(
                out=red[:, 2 * F:2 * F + 1],
                in_=pall[:].rearrange("g (d c) -> g d c", c=NC)[:, 2 * F:2 * F + 1, :],
                axis=mybir.AxisListType.X, op=ALU.add)
            nc.vector.tensor_reduce(
                out=red[:, F:2 * F],
                in_=pall[:].rearrange("g (d c) -> g d c", c=NC)[:, F:2 * F, :],
                axis=mybir.AxisListType.X, op=ALU.max)
            rc = ps.tile([64, 1], F32)
            nc.vector.reciprocal(out=rc[:], in_=red[:, 2 * F:2 * F + 1])
            zmean = ps.tile([64, F], F32)
            nc.vector.tensor_scalar_mul(out=zmean[:], in0=red[:, 0:F],
                                        scalar1=rc[:])
            # ---------------- MLP ----------------
            lw1a = ps.tile([P, F], F32)
            nc.sync.dma_start(out=lw1a[:], in_=lw1_d[0:F, :])
            lw1b = ps.tile([P, F], F32)
            nc.sync.dma_start(out=lw1b[:], in_=lw1_d[F:2 * F, :])
            lw2_sb = ps.tile([P, 16], F32)
            nc.sync.dma_start(out=lw2_sb[:], in_=lw2_d[:, :])

            zTa_ps = ps_tr.tile([P, P], F32, tag="trf")
            nc.tensor.transpose(zTa_ps[:, 0:64], zmean[:], ident_f[0:64, 0:64])
            zTa = ps.tile([P, 64], F32)
            nc.scalar.activation(zTa[:], zTa_ps[:, 0:64], AF.Copy)
            zTb_ps = ps_tr.tile([P, P], F32, tag="trf")
            nc.tensor.transpose(zTb_ps[:, 0:64], red[:, F:2 * F],
                                ident_f[0:64, 0:64])
            zTb = ps.tile([P, 64], F32)
            nc.scalar.activation(zTb[:], zTb_ps[:, 0:64], AF.Copy)
            y1_ps = ps_misc.tile([P, 512], F32, tag="misc")
            nc.tensor.matmul(y1_ps[0:64, 0:F], lhsT=zTa[:], rhs=lw1a[:],
                             start=True, stop=False, skip_group_check=True)
            nc.tensor.matmul(y1_ps[0:64, 0:F], lhsT=zTb[:], rhs=lw1b[:],
                             start=False, stop=False, skip_group_check=True)
            nc.tensor.matmul(y1_ps[0:64, 0:F], lhsT=ones_row[:, 0:64],
                             rhs=lb1row[:], start=False, stop=True,
                             skip_group_check=True)
            y1 = ps.tile([64, F], F32)
            nc.scalar.activation(y1[:], y1_ps[0:64, 0:F], AF.Relu)
            yT_ps = ps_tr.tile([P, P], F32, tag="trf")
            nc.tensor.transpose(yT_ps[:, 0:64], y1[:], ident_f[0:64, 0:64])
            yT = ps.tile([P, 64], F32)
            nc.scalar.activation(yT[:], yT_ps[:, 0:64], AF.Copy)
            o_ps = ps_misc.tile([64, 16], F32, tag="misc")
            nc.tensor.matmul(o_ps[:, :], lhsT=yT[:], rhs=lw2_sb[:],
                             start=True, stop=False, skip_group_check=True)
            nc.tensor.matmul(o_ps[:, :], lhsT=ones_row[:, 0:64], rhs=lb2row[:],
                             start=False, stop=True, skip_group_check=True)
            o_sb = ps.tile([64, 16], F32)
            nc.scalar.activation(o_sb[:], o_ps[:, :], AF.Copy)
            nc.sync.dma_start(out=out_d[:, :], in_=o_sb[:])

    nc.compile()
    return nc


_CACHE = {}


def _get_program(key, cfg):
    if key not in _CACHE:
        _CACHE[key] = _build(cfg)
    return _CACHE[key]


def kernel(x, edge_index, batch, W1, b1, W2, b2, W3, b3, W4, b4,
           g1, be1, g2, be2, g3, be3, lw1, lb1, lw2, lb2):
    x = np.asarray(x)
    cfg, percore = _prep(x, edge_index, batch)
    C = int(lw2.shape[1])

    Wstack = np.stack([np.asarray(w, np.float32) for w in (W1, W2, W3, W4)]
                      ).astype(BF16NP)
    gam = np.stack([np.asarray(g, np.float32) for g in (g1, g2, g3)])
    bet = np.stack([np.asarray(b, np.float32) for b in (be1, be2, be3)])
    lw2p = np.zeros((lw2.shape[0], 16), np.float32)
    lw2p[:, :C] = np.asarray(lw2, np.float32)
    lb2p = np.zeros((1, 16), np.float32)
    lb2p[0, :C] = np.asarray(lb2, np.float32)

    shared = dict(
        W=Wstack,
        lw1=np.asarray(lw1, np.float32),
        lw2=lw2p,
        b4=np.asarray(b4, np.float32).reshape(1, -1),
        gamma=gam, beta=bet,
        lb1=np.asarray(lb1, np.float32).reshape(1, -1),
        lb2=lb2p,
    )
    in_maps = []
    for c in range(NC):
        m = {k: v[c] for k, v in percore.items()}
        m.update(shared)
        in_maps.append(m)

    key = (cfg["B"], cfg["K"], cfg["NTOT"], cfg["G"], tuple(sorted(BISECT)))
    nc = _get_program(key, cfg)

    global LAST_RESULTS
    if SIM:
        from concourse.bass_interp import MultiCoreSim
        sim = MultiCoreSim(nc, NC)
        for c in range(NC):
            for name, arr in in_maps[c].items():
                sim.cores[c].tensor(name)[:] = arr
        sim.simulate(check_with_hw=False)
        out = np.array(sim.cores[0].mem_tensor("out"))
        LAST_RESULTS = {"exec_time_ns": None}
        return out[:, :C].copy()

    from concourse import bass_utils
    if PROFILE:
        _install_ntff_hook_shim()
    res = bass_utils.run_bass_kernel_spmd(
        nc, in_maps, list(range(NC)), trace=PROFILE)
    LAST_RESULTS = {"exec_time_ns": res.exec_time_ns,
                    "mean_exec_time_ns": res.mean_exec_time_ns}
    return res.results[0]["out"][:, :C].copy()



# revision 4
# speedup vs baseline: 1.3317x; 1.3317x over previous
"""GCN (4-layer GCNConv + BN + global mean/max pool + MLP) on 8 Trainium2 cores.

Strategy (node sharding, per the graph-partitioning hint):
  - Nodes are re-laid-out graph-aligned: each core owns 8 of the 64 graphs,
    each graph padded to a multiple of 128 node slots, so every 128-node SBUF
    block belongs to exactly one graph.
  - Each core owns every edge whose dst lands in its node range.  Edges are
    host-sorted by (dst block, src half) and padded to a uniform per-(block,
    half) chunk cap so all 8 cores run one SPMD program.
  - Per layer: h@W + dinv row-scale produce the scaled feature table u
    (bf16); an AllGather replicates the table; per-edge rows are fetched with
    dma_gather; per-128-edge chunks are summed into per-block PSUM via a
    one-hot (dst==node) matmul; BatchNorm stats go through a tiny AllGather.
  - GCN normalization uses the identity
        conv = dinv * (sum_{e->n} u[src] + u[n]) + b,   u = dinv * (h @ W)
    so no per-edge norm multiply is needed.
  - Pooling: per-block one-hot(graph) matmul for mean+counts; masked
    block-max reduction for max; one AllGather combines across cores; the
    tiny MLP runs redundantly on every core.
"""

import os
import numpy as np
import ml_dtypes

import concourse.bass as bass
import concourse.bacc as bacc
import concourse.tile as tile
from concourse import mybir
from concourse.masks import make_identity

P = 128
NC = 8
EPS = 1e-5
F32 = mybir.dt.float32
BF16 = mybir.dt.bfloat16
I16 = mybir.dt.int16
I32 = mybir.dt.int32
AF = mybir.ActivationFunctionType
ALU = mybir.AluOpType
BF16NP = ml_dtypes.bfloat16

PROFILE = False
SIM = False  # run MultiCoreSim instead of hardware (dev only)
TAPS = False  # dump per-layer intermediates (dev only)
BISECT = set()  # dev: {"noagg"} drop gather/M path, {"nocoll"} drop collectives
LAST_RESULTS = {}


def _install_ntff_hook_shim():
    """Provide antenv.axon_hooks when the agent image lacks it, so
    run_bass_kernel_spmd(trace=True) can capture NTFF profiles."""
    import sys
    import types
    import ctypes
    import contextlib
    if "antenv.axon_hooks" in sys.modules:
        return
    so = "/opt/axon/libaxon_pjrt.so"
    if not os.path.exists(so):
        return
    try:
        lib = ctypes.CDLL(so)
    except OSError:
        return
    if not hasattr(lib, "axon_start_nrt_profile"):
        return
    lib.axon_start_nrt_profile.argtypes = [
        ctypes.POINTER(ctypes.c_int64), ctypes.c_size_t]
    lib.axon_start_nrt_profile.restype = ctypes.c_int64
    lib.axon_stop_nrt_profile.argtypes = [ctypes.c_char_p]
    lib.axon_stop_nrt_profile.restype = ctypes.c_int64

    @contextlib.contextmanager
    def _hook(output_dir, device_ids):
        import jax
        jax.devices()
        if device_ids:
            ids = (ctypes.c_int64 * len(device_ids))(*device_ids)
            rc = lib.axon_start_nrt_profile(ids, len(device_ids))
        else:
            rc = lib.axon_start_nrt_profile(None, 0)
        if rc != 0:
            raise RuntimeError(f"axon_start_nrt_profile rc={rc}")
        try:
            yield
        finally:
            n = lib.axon_stop_nrt_profile(str(output_dir).encode())
            if n <= 0:
                print(f"ntff profile: {n} files written", flush=True)

    mod = types.ModuleType("antenv.axon_hooks")
    mod.get_axon_ntff_profile_hook = lambda: _hook
    mod.set_axon_ntff_profile_hook = lambda h: None
    sys.modules["antenv.axon_hooks"] = mod


# ----------------------------------------------------------------------------
# Host-side preprocessing: integer graph partitioning / layout only.
# ----------------------------------------------------------------------------
def _prep(x, edge_index, batch):
    N, F = x.shape
    E = edge_index.shape[1]
    G = 64  # fixed number of graph segments in this problem
    GPC = (G + NC - 1) // NC

    batch = np.asarray(batch).astype(np.int64)
    src = np.asarray(edge_index[0]).astype(np.int64)
    dst = np.asarray(edge_index[1]).astype(np.int64)

    counts = np.bincount(batch, minlength=G)
    nb = np.maximum((counts + P - 1) // P, 0)  # blocks per graph
    core_of_g = np.minimum(np.arange(G) // GPC, NC - 1)
    B = 0
    for c in range(NC):
        B = max(B, int(nb[core_of_g == c].sum()))
    NODES_PC = B * P
    NTOT = NC * NODES_PC
    HALF = NTOT // 2
    assert HALF < 32768, f"gather index range too large: {HALF}"

    # stable order of nodes by graph (batch is sorted in the reference, but
    # don't rely on it)
    perm = np.argsort(batch, kind="stable")
    cum = np.concatenate([[0], np.cumsum(counts)])

    # new node id for each old node
    old2new = np.empty(N, dtype=np.int64)
    blk_graph = np.full((NC, B), -1, dtype=np.int64)  # graph id per block
    for c in range(NC):
        off = 0  # block offset within core
        for g in range(c * GPC, min((c + 1) * GPC, G)):
            n_g = int(counts[g])
            ids = perm[cum[g]:cum[g] + n_g]
            base = c * NODES_PC + off * P
            old2new[ids] = base + np.arange(n_g)
            blk_graph[c, off:off + int(nb[g])] = g
            off += int(nb[g])

    # per-node arrays in new layout
    x_new = np.zeros((NTOT, F), dtype=np.float32)
    x_new[old2new] = np.asarray(x, dtype=np.float32)
    deg = np.bincount(dst, minlength=N).astype(np.float32)
    deg_new = np.zeros(NTOT, dtype=np.float32)
    deg_new[old2new] = deg
    bid_new = np.full(NTOT, -1.0, dtype=np.float32)
    bid_new[old2new] = batch.astype(np.float32)
    rm_new = np.zeros(NTOT, dtype=np.float32)
    rm_new[old2new] = 1.0

    def per_core_pb(a):  # [NTOT] -> [NC][128, B]
        return a.reshape(NC, B, P).transpose(0, 2, 1).copy()

    # graphmask [NC][1, G*B]: 1 if block b of core c belongs to graph g
    gmask = np.zeros((NC, 1, G * B), dtype=np.float32)
    for c in range(NC):
        for b in range(B):
            g = blk_graph[c, b]
            if g >= 0:
                gmask[c, 0, g * B + b] = 1.0

    # ---- edges ----
    src_n = old2new[src]
    dst_n = old2new[dst]
    core_e = dst_n // NODES_PC
    lb = (dst_n % NODES_PC) // P
    dl = dst_n % P
    half = (src_n >= HALF).astype(np.int64)
    idxv = (src_n - half * HALF).astype(np.int64)

    key = (core_e * B + lb) * 2 + half
    order = np.argsort(key, kind="stable")
    key_s = key[order]
    idx_s = idxv[order]
    dl_s = dl[order]
    grp_cnt = np.bincount(key_s, minlength=NC * B * 2)
    K = int((grp_cnt.max() + P - 1) // P)
    CAP = K * P

    idx_pad = np.zeros((NC * B * 2, CAP), dtype=np.int16)
    dl_pad = np.full((NC * B * 2, CAP), -1.0, dtype=np.float32)
    starts = np.concatenate([[0], np.cumsum(grp_cnt)])
    flat_pos = (np.arange(E) - starts[key_s]) + key_s * CAP
    idx_pad.reshape(-1)[flat_pos] = idx_s.astype(np.int16)
    dl_pad.reshape(-1)[flat_pos] = dl_s.astype(np.float32)

    idx_pad = idx_pad.reshape(NC, B, 2, CAP)
    dl_pad = dl_pad.reshape(NC, B, 2, CAP)

    def wrap_idx(stream):  # [L] int16 -> [128, L//16]
        L = stream.shape[0]
        w = stream.reshape(L // 16, 16).T
        return np.tile(w, (8, 1)).copy()

    idx_w = np.empty((NC, 2, P, B * CAP // 16), dtype=np.int16)
    dl_t = np.empty((NC, 2, P, B * K), dtype=BF16NP)
    for c in range(NC):
        for h in range(2):
            stream = idx_pad[c, :, h, :].reshape(-1)
            idx_w[c, h] = wrap_idx(stream)
            dl_t[c, h] = dl_pad[c, :, h, :].reshape(B * K, P).T.astype(BF16NP)

    cfg = dict(N=N, F=F, G=G, B=B, K=K, NODES_PC=NODES_PC, NTOT=NTOT, HALF=HALF)
    percore = dict(
        x=[x_new.reshape(NC, NODES_PC, F)[c].astype(BF16NP) for c in range(NC)],
        deg=list(per_core_pb(deg_new)),
        batchid=list(per_core_pb(bid_new)),
        realmask=list(per_core_pb(rm_new)),
        graphmask=[gmask[c] for c in range(NC)],
        idx_lo=[idx_w[c, 0] for c in range(NC)],
        idx_hi=[idx_w[c, 1] for c in range(NC)],
        dstl_lo=[dl_t[c, 0] for c in range(NC)],
        dstl_hi=[dl_t[c, 1] for c in range(NC)],
    )
    return cfg, percore


# ----------------------------------------------------------------------------
# Bass program
# ----------------------------------------------------------------------------
def _build(cfg, GRP=4):
    B, K, G = cfg["B"], cfg["K"], cfg["G"]
    NODES_PC, NTOT, HALF = cfg["NODES_PC"], cfg["NTOT"], cfg["HALF"]
    F = cfg["F"]
    NREAL = cfg["N"]  # real node count for BN mean
    CAP = K * P
    L = B * CAP  # edge slots per half

    nc = bacc.Bacc("TRN2", target_bir_lowering=False, debug=False,
                   num_devices=NC, num_swdge_queues=4)

    din = {}
    def dram_in(name, shape, dt):
        din[name] = nc.dram_tensor(name, shape, dt, kind="ExternalInput")
        return din[name]

    x_d = dram_in("x", [NODES_PC, F], BF16)
    deg_d = dram_in("deg", [P, B], F32)
    bid_d = dram_in("batchid", [P, B], F32)
    rm_d = dram_in("realmask", [P, B], F32)
    gm_d = dram_in("graphmask", [1, G * B], F32)
    ixlo_d = dram_in("idx_lo", [P, L // 16], I16)
    ixhi_d = dram_in("idx_hi", [P, L // 16], I16)
    dllo_d = dram_in("dstl_lo", [P, B * K], BF16)
    dlhi_d = dram_in("dstl_hi", [P, B * K], BF16)
    W_d = dram_in("W", [4, F, F], BF16)
    lw1_d = dram_in("lw1", [2 * F, F], F32)
    lw2_d = dram_in("lw2", [F, 16], F32)  # padded to 16 cols
    b4_d = dram_in("b4", [1, F], F32)
    gam_d = dram_in("gamma", [3, F], F32)
    bet_d = dram_in("beta", [3, F], F32)
    lb1_d = dram_in("lb1", [1, F], F32)
    lb2_d = dram_in("lb2", [1, 16], F32)
    out_d = nc.dram_tensor("out", [64, 16], F32, kind="ExternalOutput")
    taps = {}
    if TAPS:
        for k in range(4):
            taps[f"tap_u{k}"] = nc.dram_tensor(
                f"tap_u{k}", [P, B * F], BF16, kind="ExternalOutput")
            taps[f"tap_conv{k}"] = nc.dram_tensor(
                f"tap_conv{k}", [P, B * F], F32, kind="ExternalOutput")
            taps[f"tap_h{k}"] = nc.dram_tensor(
                f"tap_h{k}", [P, B * F], F32, kind="ExternalOutput")
        for k in range(3):
            taps[f"tap_stats{k}"] = nc.dram_tensor(
                f"tap_stats{k}", [NC, 2 * F], F32, kind="ExternalOutput")
            taps[f"tap_coef{k}"] = nc.dram_tensor(
                f"tap_coef{k}", [P, 2 * F], F32, kind="ExternalOutput")
        taps["tap_dinv"] = nc.dram_tensor(
            "tap_dinv", [P, B], F32, kind="ExternalOutput")
        taps["tap_part"] = nc.dram_tensor(
            "tap_part", [64, 2 * F + 1], F32, kind="ExternalOutput")

    NGRP = (B + GRP - 1) // GRP

    with tile.TileContext(nc) as tc:
        with (
            tc.tile_pool(name="dram", bufs=1, space="DRAM") as dram,
            tc.tile_pool(name="persist", bufs=1) as ps,
            tc.tile_pool(name="work", bufs=2) as wk,
            tc.tile_pool(name="hTp", bufs=3) as hTp,
            tc.tile_pool(name="gbufp", bufs=2) as gbufp,
            tc.tile_pool(name="mbufp", bufs=2) as mbufp,
            tc.tile_pool(name="ps_tr", bufs=1, space="PSUM") as ps_tr,
            tc.tile_pool(name="ps_t", bufs=1, space="PSUM") as ps_t,
            tc.tile_pool(name="ps_agg", bufs=2, space="PSUM") as ps_agg,
            tc.tile_pool(name="ps_stats", bufs=1, space="PSUM") as ps_stats,
            tc.tile_pool(name="ps_pool", bufs=1, space="PSUM") as ps_pool,
            tc.tile_pool(name="ps_misc", bufs=1, space="PSUM") as ps_misc,
        ):
            # --- internal DRAM ---
            u_own = dram.tile([NODES_PC, F], BF16)
            tables = [dram.tile([NTOT, F], BF16, addr_space="Shared",
                                name=f"table{j}", tag=f"table{j}")
                      for j in range(4)]
            table_hi = dram.tile([HALF, F], BF16)
            stats_in = dram.tile([1, 2 * F], F32)
            stats_outs = [dram.tile([NC, 2 * F], F32, addr_space="Shared",
                                    name=f"stats_out{j}", tag=f"stats_out{j}")
                          for j in range(3)]
            pool_in = dram.tile([64, 2 * F + 1], F32)
            pool_out = dram.tile([NC * 64, 2 * F + 1], F32, addr_space="Shared")

            # --- persistent SBUF ---
            ident_b = ps.tile([P, P], BF16)
            make_identity(nc, ident_b[:])
            ident_f = ps.tile([P, P], F32)
            make_identity(nc, ident_f[:])
            iota_i = ps.tile([P, P], I32)
            nc.gpsimd.iota(iota_i[:], pattern=[[1, P]], base=0,
                           channel_multiplier=0)
            iota_b = ps.tile([P, P], BF16)
            nc.vector.tensor_copy(iota_b[:], iota_i[:])
            giota_i = ps.tile([P, 64], I32)
            nc.gpsimd.iota(giota_i[:], pattern=[[1, 64]], base=0,
                           channel_multiplier=0)
            giota_f = ps.tile([P, 64], F32)
            nc.vector.tensor_copy(giota_f[:], giota_i[:])
            ones_row = ps.tile([1, P], F32)
            nc.vector.memset(ones_row[:], 1.0)
            ones8 = ps.tile([NC, 1], F32)
            nc.vector.memset(ones8[:], 1.0)

            deg_sb = ps.tile([P, B], F32)
            nc.sync.dma_start(out=deg_sb[:], in_=deg_d[:, :])
            dinv = ps.tile([P, B], F32)
            nc.vector.tensor_scalar_add(out=dinv[:], in0=deg_sb[:], scalar1=1.0)
            nc.vector.reciprocal(out=dinv[:], in_=dinv[:])
            nc.scalar.activation(dinv[:], dinv[:], AF.Sqrt)
            if TAPS:
                nc.sync.dma_start(out=taps["tap_dinv"][:, :], in_=dinv[:])
            bid_sb = ps.tile([P, B], F32)
            nc.sync.dma_start(out=bid_sb[:], in_=bid_d[:, :])
            rm_sb = ps.tile([P, B], F32)
            nc.sync.dma_start(out=rm_sb[:], in_=rm_d[:, :])
            gm_sb = ps.tile([1, G * B], F32)
            nc.sync.dma_start(out=gm_sb[:], in_=gm_d[:, :])
            ixlo_sb = ps.tile([P, L // 16], I16)
            nc.sync.dma_start(out=ixlo_sb[:], in_=ixlo_d[:, :])
            ixhi_sb = ps.tile([P, L // 16], I16)
            nc.sync.dma_start(out=ixhi_sb[:], in_=ixhi_d[:, :])
            dllo_sb = ps.tile([P, B * K], BF16)
            nc.sync.dma_start(out=dllo_sb[:], in_=dllo_d[:, :])
            dlhi_sb = ps.tile([P, B * K], BF16)
            nc.sync.dma_start(out=dlhi_sb[:], in_=dlhi_d[:, :])
            b4row = ps.tile([1, F], F32)
            nc.sync.dma_start(out=b4row[:], in_=b4_d[:, :])
            lb1row = ps.tile([1, F], F32)
            nc.sync.dma_start(out=lb1row[:], in_=lb1_d[:, :])
            lb2row = ps.tile([1, 16], F32)
            nc.sync.dma_start(out=lb2row[:], in_=lb2_d[:, :])

            h_sb = ps.tile([P, B * F], BF16)
            nc.sync.dma_start(
                out=h_sb[:].rearrange("p (b f) -> p b f", b=B),
                in_=x_d[:, :].rearrange("(b p) f -> p b f", p=P),
            )
            conv_sb = ps.tile([P, B * F], F32)
            u_sb = ps.tile([P, B * F], BF16)

            # b4 broadcast [128, F] via outer product
            b4_ps = ps_misc.tile([P, 512], F32, tag="misc")
            nc.tensor.matmul(b4_ps[:, 0:F], lhsT=ones_row[:], rhs=b4row[:],
                             start=True, stop=True)
            b4_bc = ps.tile([P, F], F32)
            nc.scalar.activation(b4_bc[:], b4_ps[:, 0:F], AF.Copy)

            # ---------------- layers ----------------
            for k in range(4):
                W_sb = wk.tile([P, F], BF16, tag="W")
                nc.sync.dma_start(out=W_sb[:], in_=W_d[k, :, :])

                # phase A: u = dinv * (h @ W) per block; stage into u_sb + DRAM
                for b in range(B):
                    hT_ps = ps_tr.tile([P, P], BF16, tag="trps")
                    nc.tensor.transpose(hT_ps[:], h_sb[:, b * F:(b + 1) * F],
                                        ident_b[:])
                    hT = hTp.tile([P, P], BF16, tag="hT")
                    nc.scalar.activation(hT[:], hT_ps[:], AF.Copy)
                    t_ps = ps_t.tile([P, F], F32, tag="tps")
                    nc.tensor.matmul(t_ps[:], lhsT=hT[:], rhs=W_sb[:],
                                     start=True, stop=True)
                    nc.scalar.activation(u_sb[:, b * F:(b + 1) * F], t_ps[:],
                                         AF.Copy, scale=dinv[:, b:b + 1])
                    nc.sync.dma_start(out=u_own[b * P:(b + 1) * P, :],
                                      in_=u_sb[:, b * F:(b + 1) * F])

                if TAPS:
                    nc.sync.dma_start(out=taps[f"tap_u{k}"][:, :], in_=u_sb[:])
                # phase B: replicate the table
                table = tables[k]
                if "nocoll" in BISECT:
                    nc.sync.dma_start(out=table[0:NODES_PC, :], in_=u_own[:, :])
                else:
                    nc.gpsimd.collective_compute(
                        "AllGather", ALU.bypass,
                        replica_groups=[list(range(NC))],
                        ins=[u_own[:, :].opt()],
                        outs=[table[:, :].opt()],
                    )
                # dma_gather ignores in_ap row offsets on HW; stage the upper
                # half at a base address of its own
                nc.sync.dma_start(out=table_hi[:, :], in_=table[HALF:NTOT, :])

                # phase C: aggregate into conv
                if k < 3:
                    st_mu = ps_stats.tile([1, F], F32, tag="stats")
                    st_e2 = ps_pool.tile([1, F], F32, tag="pool")
                for g in range(NGRP):
                    b0 = g * GRP
                    nb = min(GRP, B - b0)
                    nidx = nb * CAP
                    if "noagg" in BISECT:
                        for bb in range(nb):
                            b = b0 + bb
                            agg = ps_agg.tile([P, F], F32, tag="agg")
                            nc.tensor.matmul(agg[:], lhsT=ident_b[:],
                                             rhs=u_sb[:, b * F:(b + 1) * F],
                                             start=True, stop=True)
                            nc.scalar.activation(conv_sb[:, b * F:(b + 1) * F],
                                                 agg[:], AF.Copy,
                                                 scale=dinv[:, b:b + 1])
                            if k < 3:
                                sq = wk.tile([P, F], F32, tag="sq")
                                nc.vector.tensor_tensor(
                                    out=sq[:], in0=conv_sb[:, b * F:(b + 1) * F],
                                    in1=conv_sb[:, b * F:(b + 1) * F], op=ALU.mult)
                                nc.tensor.matmul(st_mu[:, :], lhsT=rm_sb[:, b:b + 1],
                                                 rhs=conv_sb[:, b * F:(b + 1) * F],
                                                 start=(b == 0), stop=(b == B - 1),
                                                 skip_group_check=True)
                                nc.tensor.matmul(st_e2[:, :],
                                                 lhsT=rm_sb[:, b:b + 1], rhs=sq[:],
                                                 start=(b == 0), stop=(b == B - 1),
                                                 skip_group_check=True)
                        continue
                    gl = gbufp.tile([P, nb * K, P], BF16, tag="gl")
                    gh = gbufp.tile([P, nb * K, P], BF16, tag="gh")
                    if "nogather" in BISECT:
                        nc.vector.memset(gl[:], 0.0)
                        nc.vector.memset(gh[:], 0.0)
                    else:
                        nc.gpsimd.dma_gather(
                            out_ap=gl[:], in_ap=table[0:HALF, :],
                            idxs_ap=ixlo_sb[:, b0 * CAP // 16:(b0 * CAP + nidx) // 16],
                            num_idxs=nidx, num_idxs_reg=nidx, elem_size=F,
                            single_packet=False, queue_num=(2 * g) % 4)
                        nc.gpsimd.dma_gather(
                            out_ap=gh[:], in_ap=table_hi[:, :],
                            idxs_ap=ixhi_sb[:, b0 * CAP // 16:(b0 * CAP + nidx) // 16],
                            num_idxs=nidx, num_idxs_reg=nidx, elem_size=F,
                            single_packet=False, queue_num=(2 * g + 1) % 4)
                    ml = mbufp.tile([P, nb * K, P], BF16, tag="ml")
                    mh = mbufp.tile([P, nb * K, P], BF16, tag="mh")
                    nc.vector.tensor_tensor(
                        out=ml[:], in0=dllo_sb[:, b0 * K:(b0 + nb) * K]
                        .to_broadcast([P, nb * K, P]),
                        in1=iota_b[:].rearrange("p (o f) -> p o f", o=1)
                        .to_broadcast([P, nb * K, P]),
                        op=ALU.is_equal)
                    nc.vector.tensor_tensor(
                        out=mh[:], in0=dlhi_sb[:, b0 * K:(b0 + nb) * K]
                        .to_broadcast([P, nb * K, P]),
                        in1=iota_b[:].rearrange("p (o f) -> p o f", o=1)
                        .to_broadcast([P, nb * K, P]),
                        op=ALU.is_equal)
                    for bb in range(nb):
                        b = b0 + bb
                        agg = ps_agg.tile([P, F], F32, tag="agg")
                        for j in range(K):
                            nc.tensor.matmul(
                                agg[:], lhsT=ml[:, bb * K + j, :],
                                rhs=gl[:, bb * K + j, :],
                                start=(j == 0), stop=False)
                        for j in range(K):
                            nc.tensor.matmul(
                                agg[:], lhsT=mh[:, bb * K + j, :],
                                rhs=gh[:, bb * K + j, :],
                                start=False, stop=False)
                        # self loop term
                        nc.tensor.matmul(agg[:], lhsT=ident_b[:],
                                         rhs=u_sb[:, b * F:(b + 1) * F],
                                         start=False, stop=True)
                        nc.scalar.activation(conv_sb[:, b * F:(b + 1) * F],
                                             agg[:], AF.Copy,
                                             scale=dinv[:, b:b + 1])
                        if k < 3:
                            sq = wk.tile([P, F], F32, tag="sq")
                            nc.vector.tensor_tensor(
                                out=sq[:], in0=conv_sb[:, b * F:(b + 1) * F],
                                in1=conv_sb[:, b * F:(b + 1) * F], op=ALU.mult)
                            nc.tensor.matmul(st_mu[:, :], lhsT=rm_sb[:, b:b + 1],
                                             rhs=conv_sb[:, b * F:(b + 1) * F],
                                             start=(b == 0), stop=(b == B - 1),
                                             skip_group_check=True)
                            nc.tensor.matmul(st_e2[:, :],
                                             lhsT=rm_sb[:, b:b + 1], rhs=sq[:],
                                             start=(b == 0), stop=(b == B - 1),
                                             skip_group_check=True)

                if TAPS:
                    nc.sync.dma_start(out=taps[f"tap_conv{k}"][:, :], in_=conv_sb[:])
                if k < 3:
                    # phase D: BN stats allreduce (via allgather) + apply + relu
                    st_sb = wk.tile([1, 2 * F], F32, tag="strow")
                    nc.scalar.activation(st_sb[:, 0:F], st_mu[:, :], AF.Copy)
                    nc.scalar.activation(st_sb[:, F:2 * F], st_e2[:, :], AF.Copy)
                    nc.sync.dma_start(out=stats_in[:, :], in_=st_sb[:])
                    stats_out = stats_outs[k]
                    if "nocoll" in BISECT:
                        nc.sync.dma_start(out=stats_out[0:1, :], in_=stats_in[:, :])
                    else:
                        nc.gpsimd.collective_compute(
                            "AllGather", ALU.bypass,
                            replica_groups=[list(range(NC))],
                            ins=[stats_in[:, :].opt()],
                            outs=[stats_out[:, :].opt()],
                        )
                    srows = wk.tile([NC, 2 * F], F32, tag="srows")
                    nc.sync.dma_start(out=srows[:], in_=stats_out[:, :])
                    if TAPS:
                        nc.sync.dma_start(out=taps[f"tap_stats{k}"][:, :],
                                          in_=srows[:])
                    tot_ps = ps_misc.tile([P, 512], F32, tag="misc")
                    nc.tensor.matmul(tot_ps[0:1, 0:2 * F], lhsT=ones8[:],
                                     rhs=srows[:], start=True, stop=True)
                    # mean | E[x^2]
                    mrow = wk.tile([1, 2 * F], F32, tag="mrow")
                    nc.scalar.activation(mrow[:], tot_ps[0:1, 0:2 * F],
                                         AF.Copy, scale=1.0 / NREAL)
                    coef = wk.tile([1, 2 * F], F32, tag="coef")
                    # var = E[x^2] - mean^2  -> coef[0:F] (temp)
                    nc.vector.tensor_tensor(out=coef[:, 0:F],
                                            in0=mrow[:, 0:F], in1=mrow[:, 0:F],
                                            op=ALU.mult)
                    nc.vector.tensor_tensor(out=coef[:, 0:F],
                                            in0=mrow[:, F:2 * F],
                                            in1=coef[:, 0:F], op=ALU.subtract)
                    # rs = rsqrt(var + eps)
                    nc.vector.tensor_scalar_add(out=coef[:, 0:F],
                                                in0=coef[:, 0:F], scalar1=EPS)
                    nc.vector.reciprocal(out=coef[:, 0:F], in_=coef[:, 0:F])
                    nc.scalar.activation(coef[:, 0:F], coef[:, 0:F], AF.Sqrt)
                    grow = wk.tile([1, F], F32, tag="grow")
                    nc.sync.dma_start(out=grow[:], in_=gam_d[k:k + 1, :])
                    brow = wk.tile([1, F], F32, tag="brow")
                    nc.sync.dma_start(out=brow[:], in_=bet_d[k:k + 1, :])
                    # scale = gamma * rs
                    nc.vector.tensor_tensor(out=coef[:, 0:F], in0=coef[:, 0:F],
                                            in1=grow[:], op=ALU.mult)
                    # shift = beta - mean * scale
                    tmp = wk.tile([1, F], F32, tag="tmprow")
                    nc.vector.tensor_tensor(out=tmp[:], in0=mrow[:, 0:F],
                                            in1=coef[:, 0:F], op=ALU.mult)
                    nc.vector.tensor_tensor(out=coef[:, F:2 * F], in0=brow[:],
                                            in1=tmp[:], op=ALU.subtract)
                    cf_ps = ps_misc.tile([P, 512], F32, tag="misc")
                    nc.tensor.matmul(cf_ps[:, 0:2 * F], lhsT=ones_row[:],
                                     rhs=coef[:], start=True, stop=True)
                    cf_bc = wk.tile([P, 2 * F], F32, tag="cfbc")
                    nc.scalar.activation(cf_bc[:], cf_ps[:, 0:2 * F], AF.Copy)
                    if TAPS:
                        nc.sync.dma_start(out=taps[f"tap_coef{k}"][:, :],
                                          in_=cf_bc[:])
                    # conv = conv*scale + shift ; h = relu(conv) (bf16)
                    nc.vector.tensor_tensor(
                        out=conv_sb[:], in0=conv_sb[:],
                        in1=cf_bc[:, 0:F].rearrange("p (o f) -> p o f", o=1)
                        .to_broadcast([P, B, F]), op=ALU.mult)
                    nc.vector.tensor_tensor(
                        out=conv_sb[:], in0=conv_sb[:],
                        in1=cf_bc[:, F:2 * F].rearrange("p (o f) -> p o f", o=1)
                        .to_broadcast([P, B, F]), op=ALU.add)
                    nc.scalar.activation(h_sb[:], conv_sb[:], AF.Relu)
                    if TAPS:
                        nc.gpsimd.dma_start(out=taps[f"tap_h{k}"][:, :], in_=h_sb[:])
                else:
                    # layer 4: conv += b4 ; h4 = relu(conv) kept f32 in conv_sb
                    nc.vector.tensor_tensor(
                        out=conv_sb[:], in0=conv_sb[:],
                        in1=b4_bc[:].rearrange("p (o f) -> p o f", o=1)
                        .to_broadcast([P, B, F]), op=ALU.add)
                    nc.scalar.activation(conv_sb[:], conv_sb[:], AF.Relu)
                    if TAPS:
                        nc.sync.dma_start(out=taps[f"tap_h{k}"][:, :], in_=conv_sb[:])

            # ---------------- pooling ----------------
            bmax = ps.tile([P, B], F32)
            pool_s = ps_pool.tile([64, F], F32, tag="pool")
            pool_c = ps_stats.tile([64, 1], F32, tag="stats")
            for b in range(B):
                # mask out padding rows (relu output >= 0 so max is safe)
                nc.vector.tensor_scalar_mul(
                    out=conv_sb[:, b * F:(b + 1) * F],
                    in0=conv_sb[:, b * F:(b + 1) * F],
                    scalar1=rm_sb[:, b:b + 1])
                S = wk.tile([P, 64], F32, tag="S")
                nc.vector.tensor_tensor(
                    out=S[:], in0=bid_sb[:, b:b + 1].to_broadcast([P, 64]),
                    in1=giota_f[:], op=ALU.is_equal)
                nc.tensor.matmul(pool_s[:, :], lhsT=S[:],
                                 rhs=conv_sb[:, b * F:(b + 1) * F],
                                 start=(b == 0), stop=(b == B - 1),
                                 skip_group_check=True)
                nc.tensor.matmul(pool_c[:, :], lhsT=S[:],
                                 rhs=rm_sb[:, b:b + 1],
                                 start=(b == 0), stop=(b == B - 1),
                                 skip_group_check=True)
                trf = ps_tr.tile([P, P], F32, tag="trf")
                nc.tensor.transpose(trf[:], conv_sb[:, b * F:(b + 1) * F],
                                    ident_f[:])
                h4T = hTp.tile([P, P], F32, tag="h4T")
                nc.scalar.activation(h4T[:], trf[:], AF.Copy)
                nc.vector.tensor_reduce(out=bmax[:, b:b + 1], in_=h4T[:],
                                        axis=mybir.AxisListType.X, op=ALU.max)
            # per-graph max via masked block-max, chunked by graph groups
            gmaxT = ps.tile([P, 64], F32)
            GG = max(1, 512 // B)  # graphs per chunk so GG*B <= 512
            for g0 in range(0, G, GG):
                g1 = min(g0 + GG, G)
                w = (g1 - g0) * B
                mk_ps = ps_misc.tile([P, 512], F32, tag="misc")
                nc.tensor.matmul(mk_ps[:, 0:w], lhsT=ones_row[:],
                                 rhs=gm_sb[:, g0 * B:g1 * B],
                                 start=True, stop=True)
                mck = wk.tile([P, GG * B], F32, tag="mck")
                nc.vector.tensor_tensor(
                    out=mck[:, 0:w],
                    in0=bmax[:].rearrange("p (o b) -> p o b", o=1)
                    .to_broadcast([P, g1 - g0, B]),
                    in1=mk_ps[:, 0:w].rearrange("p (g b) -> p g b", b=B),
                    op=ALU.mult)
                nc.vector.tensor_reduce(
                    out=gmaxT[:, g0:g1],
                    in_=mck[:, 0:w].rearrange("p (g b) -> p g b", b=B),
                    axis=mybir.AxisListType.X, op=ALU.max)
            if G < 64:
                nc.vector.memset(gmaxT[:, G:64], 0.0)
            # partials -> [64, 257]: [sum | max | count]
            part = ps.tile([64, 2 * F + 1], F32)
            nc.scalar.activation(part[:, 0:F], pool_s[:, :], AF.Copy)
            nc.scalar.activation(part[:, 2 * F:2 * F + 1], pool_c[:, :],
                                 AF.Copy)
            gm_ps = ps_misc.tile([P, 512], F32, tag="misc")
            nc.tensor.transpose(gm_ps[0:64, 0:P], gmaxT[:], ident_f[:])
            nc.scalar.activation(part[:, F:2 * F], gm_ps[0:64, 0:P], AF.Copy)
            nc.sync.dma_start(out=pool_in[:, :], in_=part[:])
            if TAPS:
                nc.sync.dma_start(out=taps["tap_part"][:, :], in_=part[:])
            if "nocoll" in BISECT:
                nc.sync.dma_start(out=pool_out[0:64, :], in_=pool_in[:, :])
            else:
                nc.gpsimd.collective_compute(
                    "AllGather", ALU.bypass,
                    replica_groups=[list(range(NC))],
                    ins=[pool_in[:, :].opt()],
                    outs=[pool_out[:, :].opt()],
                )
            DP = 2 * F + 1
            pall = ps.tile([64, DP * NC], F32)
            nc.sync.dma_start(
                out=pall[:].rearrange("g (d c) -> g d c", c=NC),
                in_=pool_out[:, :].rearrange("(c g) d -> g d c", c=NC),
            )
            red = ps.tile([64, DP], F32)
            nc.vector.tensor_reduce(
                out=red[:, 0:F],
                in_=pall[:].rearrange("g (d c) -> g d c", c=NC)[:, 0:F, :],
                axis=mybir.AxisListType.X, op=ALU.add)
            nc.vector.tensor_reduce(
                out=red[:, 2 * F:2 * F + 1],
                in_=pall[:].rearrange("g (d c) -> g d c", c=NC)[:, 2 * F:2 * F + 1, :],
                axis=mybir.AxisListType.X, op=ALU.add)
            nc.vector.tensor_reduce(
                out=red[:, F:2 * F],
                in_=pall[:].rearrange("g (d c) -> g d c", c=NC)[:, F:2 * F, :],
                axis=mybir.AxisListType.X, op=ALU.max)
            rc = ps.tile([64, 1], F32)
            nc.vector.reciprocal(out=rc[:], in_=red[:, 2 * F:2 * F + 1])
            zmean = ps.tile([64, F], F32)
            nc.vector.tensor_scalar_mul(out=zmean[:], in0=red[:, 0:F],
                                        scalar1=rc[:])
            # ---------------- MLP ----------------
            lw1a = ps.tile([P, F], F32)
            nc.sync.dma_start(out=lw1a[:], in_=lw1_d[0:F, :])
            lw1b = ps.tile([P, F], F32)
            nc.sync.dma_start(out=lw1b[:], in_=lw1_d[F:2 * F, :])
            lw2_sb = ps.tile([P, 16], F32)
            nc.sync.dma_start(out=lw2_sb[:], in_=lw2_d[:, :])

            zTa_ps = ps_tr.tile([P, P], F32, tag="trf")
            nc.tensor.transpose(zTa_ps[:, 0:64], zmean[:], ident_f[0:64, 0:64])
            zTa = ps.tile([P, 64], F32)
            nc.scalar.activation(zTa[:], zTa_ps[:, 0:64], AF.Copy)
            zTb_ps = ps_tr.tile([P, P], F32, tag="trf")
            nc.tensor.transpose(zTb_ps[:, 0:64], red[:, F:2 * F],
                                ident_f[0:64, 0:64])
            zTb = ps.tile([P, 64], F32)
            nc.scalar.activation(zTb[:], zTb_ps[:, 0:64], AF.Copy)
            y1_ps = ps_misc.tile([P, 512], F32, tag="misc")
            nc.tensor.matmul(y1_ps[0:64, 0:F], lhsT=zTa[:], rhs=lw1a[:],
                             start=True, stop=False, skip_group_check=True)
            nc.tensor.matmul(y1_ps[0:64, 0:F], lhsT=zTb[:], rhs=lw1b[:],
                             start=False, stop=False, skip_group_check=True)
            nc.tensor.matmul(y1_ps[0:64, 0:F], lhsT=ones_row[:, 0:64],
                             rhs=lb1row[:], start=False, stop=True,
                             skip_group_check=True)
            y1 = ps.tile([64, F], F32)
            nc.scalar.activation(y1[:], y1_ps[0:64, 0:F], AF.Relu)
            yT_ps = ps_tr.tile([P, P], F32, tag="trf")
            nc.tensor.transpose(yT_ps[:, 0:64], y1[:], ident_f[0:64, 0:64])
            yT = ps.tile([P, 64], F32)
            nc.scalar.activation(yT[:], yT_ps[:, 0:64], AF.Copy)
            o_ps = ps_misc.tile([64, 16], F32, tag="misc")
            nc.tensor.matmul(o_ps[:, :], lhsT=yT[:], rhs=lw2_sb[:],
                             start=True, stop=False, skip_group_check=True)
            nc.tensor.matmul(o_ps[:, :], lhsT=ones_row[:, 0:64], rhs=lb2row[:],
                             start=False, stop=True, skip_group_check=True)
            o_sb = ps.tile([64, 16], F32)
            nc.scalar.activation(o_sb[:], o_ps[:, :], AF.Copy)
            nc.sync.dma_start(out=out_d[:, :], in_=o_sb[:])

    nc.compile()
    return nc


_CACHE = {}


def _get_program(key, cfg):
    if key not in _CACHE:
        _CACHE[key] = _build(cfg)
    return _CACHE[key]


def kernel(x, edge_index, batch, W1, b1, W2, b2, W3, b3, W4, b4,
           g1, be1, g2, be2, g3, be3, lw1, lb1, lw2, lb2):
    x = np.asarray(x)
    cfg, percore = _prep(x, edge_index, batch)
    C = int(lw2.shape[1])

    Wstack = np.stack([np.asarray(w, np.float32) for w in (W1, W2, W3, W4)]
                      ).astype(BF16NP)
    gam = np.stack([np.asarray(g, np.float32) for g in (g1, g2, g3)])
    bet = np.stack([np.asarray(b, np.float32) for b in (be1, be2, be3)])
    lw2p = np.zeros((lw2.shape[0], 16), np.float32)
    lw2p[:, :C] = np.asarray(lw2, np.float32)
    lb2p = np.zeros((1, 16), np.float32)
    lb2p[0, :C] = np.asarray(lb2, np.float32)

    shared = dict(
        W=Wstack,
        lw1=np.asarray(lw1, np.float32),
        lw2=lw2p,
        b4=np.asarray(b4, np.float32).reshape(1, -1),
        gamma=gam, beta=bet,
        lb1=np.asarray(lb1, np.float32).reshape(1, -1),
        lb2=lb2p,
    )
    in_maps = []
    for c in range(NC):
        m = {k: v[c] for k, v in percore.items()}
        m.update(shared)
        in_maps.append(m)

    key = (cfg["B"], cfg["K"], cfg["NTOT"], cfg["G"], tuple(sorted(BISECT)))
    nc = _get_program(key, cfg)

    global LAST_RESULTS
    if SIM:
        from concourse.bass_interp import MultiCoreSim
        sim = MultiCoreSim(nc, NC)
        for c in range(NC):
            for name, arr in in_maps[c].items():
                sim.cores[c].tensor(name)[:] = arr
        sim.simulate(check_with_hw=False)
        out = np.array(sim.cores[0].mem_tensor("out"))
        LAST_RESULTS = {"exec_time_ns": None}
        return out[:, :C].copy()

    from concourse import bass_utils
    if PROFILE:
        _install_ntff_hook_shim()
    res = bass_utils.run_bass_kernel_spmd(
        nc, in_maps, list(range(NC)), trace=PROFILE)
    LAST_RESULTS = {"exec_time_ns": res.exec_time_ns,
                    "mean_exec_time_ns": res.mean_exec_time_ns}
    return res.results[0]["out"][:, :C].copy()

